# revision 26
# baseline (speedup 1.0000x reference)
"""Trainium2 Bass kernel for nn_ExtractPatchesPositionLayer.

Reference semantics: per image b, bilinear-translate the (522,522,1) padded
object by t = -positions[b] (tfa.translate: out(y,x) = img(y+py, x+px),
zero fill outside), then center-crop 5px -> (512,512,1).

The shift is constant per image, so out is a sum of 4 constant-weighted
shifted crops of the zero-margin-padded image I:

  out = (1-wy)(1-wx) I00 + (1-wy)wx I01 + wy(1-wx) I10 + wy wx I11
      = b*(a*X00 + X10) + (a*X01 + X11)        with X = (wy*wx) * I,
        a = (1-wy)/wy, b = (1-wx)/wx           (host pre-scales X by wy*wx)

Layout: each SBUF partition holds R=8 output rows (R+1 contiguous input
rows = one dynamic-offset DMA descriptor of (R+1)*wpad elements per
partition); 64 partitions per image, 2 images per 128-partition group,
16 groups per core. Each blend pass a*in0 + in1 runs as tensor_scalar
(DVE 4x packed mode with a [P,1] scalar -- scalar_tensor_tensor only has
a 1x uop) followed by tensor_tensor (DVE 2x packed mode, tolerant of the
odd +1-element column shift and strided rows; all HW-probed). The V-pass
scale runs on ACT to balance engine load. Row/col shifts are free-axis AP
offsets (wpad and 1).

Output rows land contiguously: one 128-partition store per group, fully
contiguous in DRAM. All DMA partition counts are multiples of 16 so HWDGE
spreads descriptors across all 16 SDMA engines (a 127-partition store
serializes onto one engine -- the original kernel's bottleneck, 1.42 ms).
bf16 input/output halves DMA bytes (rel err ~3e-3, tol 2e-2); loads and
stores alternate across both HWDGE rings (SP + ACT). The kernel is
HBM-bound: ~36 MB/core at the congested-HBM effective rate ~290 GB/s/core.

Sharding: batch 256 -> 32 images x 8 cores, embarrassingly parallel.
"""

from dataclasses import dataclass

import numpy as np

import concourse.bacc as bacc
import concourse.bass as bass
import concourse.mybir as mybir
import concourse.tile as tile
from concourse.bass_utils import run_bass_kernel_spmd

PAD = 5
N_CORES = 8
EPS = 1e-6


@dataclass(frozen=True)
class Cfg:
    bpc: int       # images per core
    n: int         # output height/width
    wpad: int      # padded input height/width (with zero margin)
    in_bf16: bool
    out_bf16: bool

    @property
    def r(self):   # output rows per partition
        return 8

    @property
    def ppi(self):  # partitions per image
        return self.n // self.r

    @property
    def gi(self):   # images per group
        return 128 // self.ppi


def build_nc(cfg: Cfg) -> bass.Bass:
    BPC, N, WPAD = cfg.bpc, cfg.n, cfg.wpad
    R, PPI, GI = cfg.r, cfg.ppi, cfg.gi
    NG = BPC // GI
    FIN = (R + 1) * WPAD
    TOT = BPC * WPAD * WPAD
    f32 = mybir.dt.float32
    i32 = mybir.dt.int32
    dt_in = mybir.dt.bfloat16 if cfg.in_bf16 else f32
    dt_out = mybir.dt.bfloat16 if cfg.out_bf16 else f32
    assert N == R * PPI and BPC % GI == 0 and 128 % PPI == 0
    mult = mybir.AluOpType.mult
    add = mybir.AluOpType.add

    nc = bacc.Bacc("TRN2", target_bir_lowering=False, debug=False)
    x_d = nc.declare_dram_parameter("x", [1, TOT + WPAD], dt_in, isOutput=False)
    offs_d = nc.declare_dram_parameter("offs", [1, BPC], i32, isOutput=False)
    wvec_d = nc.declare_dram_parameter("wvec", [128, 4 * NG], f32, isOutput=False)
    y_d = nc.declare_dram_parameter("y", [1, BPC * N * N], dt_out, isOutput=True)

    # Each blend pass a*in0 + in1 is decomposed into tensor_scalar (runs in
    # DVE 4x packed mode with a [P,1] scalar, unlike scalar_tensor_tensor
    # which only has a 1x uop) followed by tensor_tensor (2x packed mode,
    # tolerant of the odd +1-element shift and strided rows -- HW-probed).
    # The V-pass tensor_scalar goes to ACT (otherwise idle) to balance.

    with tile.TileContext(nc) as tc:
        with (
            tc.tile_pool(name="const", bufs=1) as constp,
            tc.tile_pool(name="xp", bufs=3) as xp,
            tc.tile_pool(name="tap", bufs=2) as tap,
            tc.tile_pool(name="vp", bufs=2) as vp,
            tc.tile_pool(name="tbp", bufs=2) as tbp,
            tc.tile_pool(name="up", bufs=2) as up,
        ):
            offs_sb = constp.tile([1, BPC], i32, tag="offs")
            nc.sync.dma_start(offs_sb[:], offs_d[:, :])
            wvec_sb = constp.tile([128, 4 * NG], f32, tag="wvec")
            nc.sync.dma_start(wvec_sb[:], wvec_d[:, :])

            nreg = 8
            pools = []
            for eng_t, eng in ((mybir.EngineType.SP, nc.sync),
                               (mybir.EngineType.Activation, nc.scalar)):
                regs = [nc.alloc_register(eng_t, f"dyn_{eng_t}_{k}")
                        for k in range(nreg)]
                svs = [nc.snap(r, donate=True, min_val=0, max_val=TOT - 1)
                       for r in regs]
                pools.append((eng, regs, svs))

            for g in range(NG):
                xt = xp.tile([128, FIN], dt_in, tag="x")
                for s in range(GI):
                    i = g * GI + s
                    eng, regs, svs = pools[i % 2]
                    k = (i // 2) % nreg
                    eng.reg_load(regs[k], offs_sb[0:1, i:i + 1])
                    eng.dma_start(
                        xt[s * PPI:(s + 1) * PPI, :],
                        bass.AP(x_d, svs[k], [[R * WPAD, PPI], [1, FIN]]),
                        single_packet=True)

                # V = a*X[r] + X[r+1]:  TA = a*X (ACT, flat), V = TA0 + X1 (DVE TT 2x)
                ta = tap.tile([128, FIN], dt_in, tag="ta")
                nc.scalar.mul(ta[:], xt[:], wvec_sb[:, 4 * g:4 * g + 1])
                x3 = xt[:].rearrange("p (r w) -> p r w", w=WPAD)
                a3 = ta[:].rearrange("p (r w) -> p r w", w=WPAD)
                vt = vp.tile([128, R * (N + 2)], dt_in, tag="v")
                v3 = vt[:].rearrange("p (r w) -> p r w", w=N + 2)
                nc.vector.tensor_tensor(
                    out=v3, in0=a3[:, 0:R, 0:N + 2],
                    in1=x3[:, 1:R + 1, 0:N + 2], op=add)

                # U = b*V[j] + V[j+1]:  TB = b*V (DVE TS 4x), U = TB0 + V1 (DVE TT 2x)
                tb = tbp.tile([128, R * (N + 2)], dt_in, tag="tb")
                nc.vector.tensor_scalar_mul(
                    tb[:], vt[:], wvec_sb[:, 4 * g + 1:4 * g + 2])
                b3 = tb[:].rearrange("p (r w) -> p r w", w=N + 2)
                ut = up.tile([128, R * N], dt_out, tag="u")
                u3 = ut[:].rearrange("p (r w) -> p r w", w=N)
                nc.vector.tensor_tensor(
                    out=u3, in0=b3[:, :, 0:N], in1=v3[:, :, 1:N + 1], op=add)

                seng = nc.scalar if g % 2 == 0 else nc.sync
                seng.dma_start(
                    bass.AP(y_d, g * GI * N * N, [[R * N, 128], [1, R * N]]),
                    ut[:], single_packet=True)
    nc.compile()
    return nc


def host_prep(padded: np.ndarray, positions: np.ndarray, n_cores: int,
              in_bf16: bool, out_bf16: bool):
    """Shard + build metadata. padded: (B, npad, npad) f32, positions: (B, 2)."""
    B, npad, _ = padded.shape
    n = npad - 2 * PAD
    bpc = B // n_cores

    px = positions[:, 0].astype(np.float32)
    py = positions[:, 1].astype(np.float32)
    fy = np.floor(py)
    fx = np.floor(px)
    ay = (5 + fy).astype(np.int64)
    ax = (5 + fx).astype(np.int64)
    wy = np.clip((py - fy).astype(np.float32), EPS, None)
    wx = np.clip((px - fx).astype(np.float32), EPS, None)

    m_lo = int(max(0, -min(ay.min(), ax.min())))
    m_hi = int(max(0, max(ay.max(), ax.max()) + (n + 1) - npad))
    wpad = npad + m_lo + m_hi

    cv = (wy * wx).astype(np.float32)
    pp = np.zeros((B, wpad, wpad), dtype=np.float32)
    # fold the wy*wx scale into the zero-pad copy (frees an on-chip op)
    pp[:, m_lo:m_lo + npad, m_lo:m_lo + npad] = \
        padded * cv[:, None, None]
    if in_bf16:
        import ml_dtypes
        pp = pp.astype(ml_dtypes.bfloat16)

    av = ((1.0 - wy) / wy).astype(np.float32)
    bv = ((1.0 - wx) / wx).astype(np.float32)
    A = ay + m_lo
    Bc = ax + m_lo

    cfg = Cfg(bpc=bpc, n=n, wpad=wpad, in_bf16=in_bf16, out_bf16=out_bf16)
    GI, PPI = cfg.gi, cfg.ppi
    NG = bpc // GI

    in_maps = []
    for cidx in range(n_cores):
        sl = slice(cidx * bpc, (cidx + 1) * bpc)
        x_flat = np.concatenate(
            [pp[sl].reshape(-1),
             np.zeros(wpad, dtype=pp.dtype)])[None, :]
        base = np.arange(bpc, dtype=np.int64) * (wpad * wpad)
        offs = (base + A[sl] * wpad + Bc[sl]).astype(np.int32)[None, :]
        wvec = np.zeros((128, 4 * NG), dtype=np.float32)
        for g in range(NG):
            for s in range(GI):
                i = cidx * bpc + g * GI + s
                prt = slice(s * PPI, (s + 1) * PPI)
                wvec[prt, 4 * g + 0] = av[i]
                wvec[prt, 4 * g + 1] = bv[i]
        in_maps.append({"x": np.ascontiguousarray(x_flat),
                        "offs": offs, "wvec": wvec})
    return cfg, in_maps


IN_BF16 = True
OUT_BF16 = True
_nc_cache: dict = {}


def kernel(padded_obj: np.ndarray, positions: np.ndarray) -> np.ndarray:
    padded_obj = np.asarray(padded_obj)
    positions = np.asarray(positions)
    B, npad, _, C = padded_obj.shape
    cfg, in_maps = host_prep(
        padded_obj.reshape(B, npad, npad).astype(np.float32, copy=False),
        positions, N_CORES, IN_BF16, OUT_BF16)

    nc = _nc_cache.get(cfg)
    if nc is None:
        nc = build_nc(cfg)
        _nc_cache[cfg] = nc

    res = run_bass_kernel_spmd(nc, in_maps, core_ids=list(range(N_CORES)))
    out = np.concatenate(
        [np.asarray(r["y"]).astype(np.float32).reshape(cfg.bpc, cfg.n, cfg.n)
         for r in res.results], axis=0)
    return out.reshape(B, cfg.n, cfg.n, 1)


# revision 29
# speedup vs baseline: 1.0554x; 1.0554x over previous
"""Trainium2 Bass kernel for nn_ExtractPatchesPositionLayer.

Reference semantics: per image b, bilinear-translate the (522,522,1) padded
object by t = -positions[b] (tfa.translate: out(y,x) = img(y+py, x+px),
zero fill outside), then center-crop 5px -> (512,512,1).

The shift is constant per image, so out is a sum of 4 constant-weighted
shifted crops of the zero-margin-padded image I:

  out = (1-wy)(1-wx) I00 + (1-wy)wx I01 + wy(1-wx) I10 + wy wx I11
      = b*(a*X00 + X10) + (a*X01 + X11)        with X = (wy*wx) * I,
        a = (1-wy)/wy, b = (1-wx)/wx           (host pre-scales X by wy*wx)

Layout: each SBUF partition holds R=8 output rows (R+1 contiguous input
rows = one dynamic-offset DMA descriptor of (R+1)*wpad elements per
partition); 64 partitions per image, 2 images per 128-partition group,
16 groups per core. Each blend pass a*in0 + in1 runs as tensor_scalar
(DVE 4x packed mode with a [P,1] scalar -- scalar_tensor_tensor only has
a 1x uop) followed by tensor_tensor (DVE 2x packed mode, tolerant of the
odd +1-element column shift and strided rows; all HW-probed). The V-pass
scale runs on ACT to balance engine load. Row/col shifts are free-axis AP
offsets (wpad and 1).

Output rows land contiguously: one 128-partition store per group, fully
contiguous in DRAM. All DMA partition counts are multiples of 16 so HWDGE
spreads descriptors across all 16 SDMA engines (a 127-partition store
serializes onto one engine -- the original kernel's bottleneck, 1.42 ms).
bf16 input/output halves DMA bytes (rel err ~3e-3, tol 2e-2); loads and
stores alternate across both HWDGE rings (SP + ACT). The kernel is
HBM-bound: ~36 MB/core at the congested-HBM effective rate ~290 GB/s/core.

Sharding: batch 256 -> 32 images x 8 cores, embarrassingly parallel.
"""

from dataclasses import dataclass

import numpy as np

import concourse.bacc as bacc
import concourse.bass as bass
import concourse.mybir as mybir
import concourse.tile as tile
from concourse.bass_utils import run_bass_kernel_spmd

PAD = 5
N_CORES = 8
EPS = 1e-6


@dataclass(frozen=True)
class Cfg:
    bpc: int       # images per core
    n: int         # output height/width
    wpad: int      # padded input height/width (with zero margin)
    in_bf16: bool
    out_bf16: bool

    @property
    def r(self):   # output rows per partition
        return 8

    @property
    def ppi(self):  # partitions per image
        return self.n // self.r

    @property
    def gi(self):   # images per group
        return 128 // self.ppi


def build_nc(cfg: Cfg) -> bass.Bass:
    BPC, N, WPAD = cfg.bpc, cfg.n, cfg.wpad
    R, PPI, GI = cfg.r, cfg.ppi, cfg.gi
    NG = BPC // GI
    FIN = (R + 1) * WPAD
    TOT = BPC * WPAD * WPAD
    f32 = mybir.dt.float32
    i32 = mybir.dt.int32
    dt_in = mybir.dt.bfloat16 if cfg.in_bf16 else f32
    dt_out = mybir.dt.bfloat16 if cfg.out_bf16 else f32
    assert N == R * PPI and BPC % GI == 0 and 128 % PPI == 0
    mult = mybir.AluOpType.mult
    add = mybir.AluOpType.add

    nc = bacc.Bacc("TRN2", target_bir_lowering=False, debug=False)
    x_d = nc.declare_dram_parameter("x", [1, TOT + WPAD], dt_in, isOutput=False)
    offs_d = nc.declare_dram_parameter("offs", [1, BPC], i32, isOutput=False)
    wvec_d = nc.declare_dram_parameter("wvec", [128, 4 * NG], f32, isOutput=False)
    y_d = nc.declare_dram_parameter("y", [1, BPC * N * N], dt_out, isOutput=True)

    # Each blend pass a*in0 + in1 is decomposed into tensor_scalar (runs in
    # DVE 4x packed mode with a [P,1] scalar, unlike scalar_tensor_tensor
    # which only has a 1x uop) followed by tensor_tensor (2x packed mode,
    # tolerant of the odd +1-element shift and strided rows -- HW-probed).
    # The V-pass tensor_scalar goes to ACT (otherwise idle) to balance.

    with tile.TileContext(nc) as tc:
        with (
            tc.tile_pool(name="const", bufs=1) as constp,
            tc.tile_pool(name="xp", bufs=3) as xp,
            tc.tile_pool(name="tap", bufs=2) as tap,
            tc.tile_pool(name="vp", bufs=2) as vp,
            tc.tile_pool(name="tbp", bufs=2) as tbp,
            tc.tile_pool(name="up", bufs=2) as up,
        ):
            offs_sb = constp.tile([1, BPC], i32, tag="offs")
            nc.sync.dma_start(offs_sb[:], offs_d[:, :])
            wvec_sb = constp.tile([128, 4 * NG], f32, tag="wvec")
            nc.sync.dma_start(wvec_sb[:], wvec_d[:, :])

            nreg = 8
            pools = []
            for eng_t, eng in ((mybir.EngineType.SP, nc.sync),
                               (mybir.EngineType.Activation, nc.scalar)):
                regs = [nc.alloc_register(eng_t, f"dyn_{eng_t}_{k}")
                        for k in range(nreg)]
                svs = [nc.snap(r, donate=True, min_val=0, max_val=TOT - 1)
                       for r in regs]
                pools.append((eng, regs, svs))

            for g in range(NG):
                xt = xp.tile([128, FIN], dt_in, tag="x")
                for s in range(GI):
                    i = g * GI + s
                    eng, regs, svs = pools[i % 2]
                    k = (i // 2) % nreg
                    eng.reg_load(regs[k], offs_sb[0:1, i:i + 1])
                    eng.dma_start(
                        xt[s * PPI:(s + 1) * PPI, :],
                        bass.AP(x_d, svs[k], [[R * WPAD, PPI], [1, FIN]]),
                        single_packet=True)

                # V = a*X[r] + X[r+1]:  TA = a*X (ACT, flat), V = TA0 + X1 (DVE TT 2x)
                ta = tap.tile([128, FIN], dt_in, tag="ta")
                nc.scalar.mul(ta[:], xt[:], wvec_sb[:, 4 * g:4 * g + 1])
                x3 = xt[:].rearrange("p (r w) -> p r w", w=WPAD)
                a3 = ta[:].rearrange("p (r w) -> p r w", w=WPAD)
                vt = vp.tile([128, R * (N + 2)], dt_in, tag="v")
                v3 = vt[:].rearrange("p (r w) -> p r w", w=N + 2)
                nc.vector.tensor_tensor(
                    out=v3, in0=a3[:, 0:R, 0:N + 2],
                    in1=x3[:, 1:R + 1, 0:N + 2], op=add)

                # U = b*V[j] + V[j+1]:  TB = b*V (DVE TS 4x), U = TB0 + V1 (DVE TT 2x)
                tb = tbp.tile([128, R * (N + 2)], dt_in, tag="tb")
                nc.vector.tensor_scalar_mul(
                    tb[:], vt[:], wvec_sb[:, 4 * g + 1:4 * g + 2])
                b3 = tb[:].rearrange("p (r w) -> p r w", w=N + 2)
                ut = up.tile([128, R * N], dt_out, tag="u")
                u3 = ut[:].rearrange("p (r w) -> p r w", w=N)
                seng = nc.scalar if g % 2 == 0 else nc.sync
                H = R // 2
                for h in range(2):
                    nc.vector.tensor_tensor(
                        out=u3[:, h * H:(h + 1) * H, :],
                        in0=b3[:, h * H:(h + 1) * H, 0:N],
                        in1=v3[:, h * H:(h + 1) * H, 1:N + 1], op=add)
                    seng.dma_start(
                        bass.AP(y_d, g * GI * N * N + h * H * N,
                                [[R * N, 128], [1, H * N]]),
                        ut[:, h * H * N:(h + 1) * H * N], single_packet=True)
    nc.compile()
    return nc


def host_prep(padded: np.ndarray, positions: np.ndarray, n_cores: int,
              in_bf16: bool, out_bf16: bool):
    """Shard + build metadata. padded: (B, npad, npad) f32, positions: (B, 2)."""
    B, npad, _ = padded.shape
    n = npad - 2 * PAD
    bpc = B // n_cores

    px = positions[:, 0].astype(np.float32)
    py = positions[:, 1].astype(np.float32)
    fy = np.floor(py)
    fx = np.floor(px)
    ay = (5 + fy).astype(np.int64)
    ax = (5 + fx).astype(np.int64)
    wy = np.clip((py - fy).astype(np.float32), EPS, None)
    wx = np.clip((px - fx).astype(np.float32), EPS, None)

    m_lo = int(max(0, -min(ay.min(), ax.min())))
    m_hi = int(max(0, max(ay.max(), ax.max()) + (n + 1) - npad))
    wpad = npad + m_lo + m_hi

    cv = (wy * wx).astype(np.float32)
    pp = np.zeros((B, wpad, wpad), dtype=np.float32)
    # fold the wy*wx scale into the zero-pad copy (frees an on-chip op)
    pp[:, m_lo:m_lo + npad, m_lo:m_lo + npad] = \
        padded * cv[:, None, None]
    if in_bf16:
        import ml_dtypes
        pp = pp.astype(ml_dtypes.bfloat16)

    av = ((1.0 - wy) / wy).astype(np.float32)
    bv = ((1.0 - wx) / wx).astype(np.float32)
    A = ay + m_lo
    Bc = ax + m_lo

    cfg = Cfg(bpc=bpc, n=n, wpad=wpad, in_bf16=in_bf16, out_bf16=out_bf16)
    GI, PPI = cfg.gi, cfg.ppi
    NG = bpc // GI

    in_maps = []
    for cidx in range(n_cores):
        sl = slice(cidx * bpc, (cidx + 1) * bpc)
        x_flat = np.concatenate(
            [pp[sl].reshape(-1),
             np.zeros(wpad, dtype=pp.dtype)])[None, :]
        base = np.arange(bpc, dtype=np.int64) * (wpad * wpad)
        offs = (base + A[sl] * wpad + Bc[sl]).astype(np.int32)[None, :]
        wvec = np.zeros((128, 4 * NG), dtype=np.float32)
        for g in range(NG):
            for s in range(GI):
                i = cidx * bpc + g * GI + s
                prt = slice(s * PPI, (s + 1) * PPI)
                wvec[prt, 4 * g + 0] = av[i]
                wvec[prt, 4 * g + 1] = bv[i]
        in_maps.append({"x": np.ascontiguousarray(x_flat),
                        "offs": offs, "wvec": wvec})
    return cfg, in_maps


IN_BF16 = True
OUT_BF16 = True
_nc_cache: dict = {}


def kernel(padded_obj: np.ndarray, positions: np.ndarray) -> np.ndarray:
    padded_obj = np.asarray(padded_obj)
    positions = np.asarray(positions)
    B, npad, _, C = padded_obj.shape
    cfg, in_maps = host_prep(
        padded_obj.reshape(B, npad, npad).astype(np.float32, copy=False),
        positions, N_CORES, IN_BF16, OUT_BF16)

    nc = _nc_cache.get(cfg)
    if nc is None:
        nc = build_nc(cfg)
        _nc_cache[cfg] = nc

    res = run_bass_kernel_spmd(nc, in_maps, core_ids=list(range(N_CORES)))
    out = np.concatenate(
        [np.asarray(r["y"]).astype(np.float32).reshape(cfg.bpc, cfg.n, cfg.n)
         for r in res.results], axis=0)
    return out.reshape(B, cfg.n, cfg.n, 1)


# revision 31
# speedup vs baseline: 1.0673x; 1.0113x over previous
"""Trainium2 Bass kernel for nn_ExtractPatchesPositionLayer.

Reference semantics: per image b, bilinear-translate the (522,522,1) padded
object by t = -positions[b] (tfa.translate: out(y,x) = img(y+py, x+px),
zero fill outside), then center-crop 5px -> (512,512,1).

The shift is constant per image, so out is a sum of 4 constant-weighted
shifted crops of the zero-margin-padded image I:

  out = (1-wy)(1-wx) I00 + (1-wy)wx I01 + wy(1-wx) I10 + wy wx I11
      = b*(a*X00 + X10) + (a*X01 + X11)        with X = (wy*wx) * I,
        a = (1-wy)/wy, b = (1-wx)/wx           (host pre-scales X by wy*wx)

Layout: each SBUF partition holds R=8 output rows (R+1 contiguous input
rows = one dynamic-offset DMA descriptor of (R+1)*wpad elements per
partition); 64 partitions per image, 2 images per 128-partition group,
16 groups per core. Each blend pass a*in0 + in1 runs as tensor_scalar
(DVE 4x packed mode with a [P,1] scalar -- scalar_tensor_tensor only has
a 1x uop) followed by tensor_tensor (DVE 2x packed mode, tolerant of the
odd +1-element column shift and strided rows; all HW-probed). The V-pass
scale runs on ACT to balance engine load. Row/col shifts are free-axis AP
offsets (wpad and 1).

Output rows land contiguously: one 128-partition store per group, fully
contiguous in DRAM. All DMA partition counts are multiples of 16 so HWDGE
spreads descriptors across all 16 SDMA engines (a 127-partition store
serializes onto one engine -- the original kernel's bottleneck, 1.42 ms).
bf16 input/output halves DMA bytes (rel err ~3e-3, tol 2e-2); loads and
stores alternate across both HWDGE rings (SP + ACT). The kernel is
HBM-bound: ~36 MB/core at the congested-HBM effective rate ~290 GB/s/core.

Sharding: batch 256 -> 32 images x 8 cores, embarrassingly parallel.
"""

from dataclasses import dataclass

import numpy as np

import concourse.bacc as bacc
import concourse.bass as bass
import concourse.mybir as mybir
import concourse.tile as tile
from concourse.bass_utils import run_bass_kernel_spmd

PAD = 5
N_CORES = 8
EPS = 1e-6


@dataclass(frozen=True)
class Cfg:
    bpc: int       # images per core
    n: int         # output height/width
    wpad: int      # padded input height/width (with zero margin)
    in_bf16: bool
    out_bf16: bool

    @property
    def r(self):   # output rows per partition
        return 8

    @property
    def ppi(self):  # partitions per image
        return self.n // self.r

    @property
    def gi(self):   # images per group
        return 128 // self.ppi


def build_nc(cfg: Cfg) -> bass.Bass:
    BPC, N, WPAD = cfg.bpc, cfg.n, cfg.wpad
    R, PPI, GI = cfg.r, cfg.ppi, cfg.gi
    NG = BPC // GI
    FIN = (R + 1) * WPAD
    TOT = BPC * WPAD * WPAD
    f32 = mybir.dt.float32
    i32 = mybir.dt.int32
    dt_in = mybir.dt.bfloat16 if cfg.in_bf16 else f32
    dt_out = mybir.dt.bfloat16 if cfg.out_bf16 else f32
    assert N == R * PPI and BPC % GI == 0 and 128 % PPI == 0
    mult = mybir.AluOpType.mult
    add = mybir.AluOpType.add

    nc = bacc.Bacc("TRN2", target_bir_lowering=False, debug=False)
    x_d = nc.declare_dram_parameter("x", [1, TOT + WPAD], dt_in, isOutput=False)
    offs_d = nc.declare_dram_parameter("offs", [1, BPC], i32, isOutput=False)
    wvec_d = nc.declare_dram_parameter("wvec", [128, 4 * NG], f32, isOutput=False)
    y_d = nc.declare_dram_parameter("y", [1, BPC * N * N], dt_out, isOutput=True)

    # Each blend pass a*in0 + in1 is decomposed into tensor_scalar (runs in
    # DVE 4x packed mode with a [P,1] scalar, unlike scalar_tensor_tensor
    # which only has a 1x uop) followed by tensor_tensor (2x packed mode,
    # tolerant of the odd +1-element shift and strided rows -- HW-probed).
    # The V-pass tensor_scalar goes to ACT (otherwise idle) to balance.

    with tile.TileContext(nc) as tc:
        with (
            tc.tile_pool(name="const", bufs=1) as constp,
            tc.tile_pool(name="xp", bufs=3) as xp,
            tc.tile_pool(name="tap", bufs=2) as tap,
            tc.tile_pool(name="vp", bufs=2) as vp,
            tc.tile_pool(name="tbp", bufs=2) as tbp,
            tc.tile_pool(name="up", bufs=2) as up,
        ):
            offs_sb = constp.tile([1, BPC], i32, tag="offs")
            nc.sync.dma_start(offs_sb[:], offs_d[:, :])
            wvec_sb = constp.tile([128, 4 * NG], f32, tag="wvec")
            nc.sync.dma_start(wvec_sb[:], wvec_d[:, :])

            nreg = 8
            pools = []
            for eng_t, eng in ((mybir.EngineType.SP, nc.sync),
                               (mybir.EngineType.Activation, nc.scalar)):
                regs = [nc.alloc_register(eng_t, f"dyn_{eng_t}_{k}")
                        for k in range(nreg)]
                svs = [nc.snap(r, donate=True, min_val=0, max_val=TOT - 1)
                       for r in regs]
                pools.append((eng, regs, svs))

            for g in range(NG):
                xt = xp.tile([128, FIN], dt_in, tag="x")
                for s in range(GI):
                    i = g * GI + s
                    eng, regs, svs = pools[i % 2]
                    k = (i // 2) % nreg
                    eng.reg_load(regs[k], offs_sb[0:1, i:i + 1])
                    eng.dma_start(
                        xt[s * PPI:(s + 1) * PPI, :],
                        bass.AP(x_d, svs[k], [[R * WPAD, PPI], [1, FIN]]),
                        single_packet=True)

                # V = a*X[r] + X[r+1]:  TA = a*X (ACT, flat), V = TA0 + X1 (DVE TT 2x)
                ta = tap.tile([128, FIN], dt_in, tag="ta")
                nc.scalar.mul(ta[:], xt[:], wvec_sb[:, 4 * g:4 * g + 1])
                x3 = xt[:].rearrange("p (r w) -> p r w", w=WPAD)
                a3 = ta[:].rearrange("p (r w) -> p r w", w=WPAD)
                vt = vp.tile([128, R * (N + 2)], dt_in, tag="v")
                v3 = vt[:].rearrange("p (r w) -> p r w", w=N + 2)
                nc.vector.tensor_tensor(
                    out=v3, in0=a3[:, 0:R, 0:N + 2],
                    in1=x3[:, 1:R + 1, 0:N + 2], op=add)

                # U = b*V[j] + V[j+1]:  TB = b*V (DVE TS 4x), U = TB0 + V1 (DVE TT 2x)
                tb = tbp.tile([128, R * (N + 2)], dt_in, tag="tb")
                nc.vector.tensor_scalar_mul(
                    tb[:], vt[:], wvec_sb[:, 4 * g + 1:4 * g + 2])
                b3 = tb[:].rearrange("p (r w) -> p r w", w=N + 2)
                ut = up.tile([128, R * N], dt_out, tag="u")
                u3 = ut[:].rearrange("p (r w) -> p r w", w=N)
                seng = nc.scalar if g % 2 == 0 else nc.sync
                H = R // 2
                for h in range(2):
                    nc.vector.tensor_tensor(
                        out=u3[:, h * H:(h + 1) * H, :],
                        in0=b3[:, h * H:(h + 1) * H, 0:N],
                        in1=v3[:, h * H:(h + 1) * H, 1:N + 1], op=add)
                    seng.dma_start(
                        bass.AP(y_d, g * GI * N * N + h * H * N,
                                [[R * N, 128], [1, H * N]]),
                        ut[:, h * H * N:(h + 1) * H * N], single_packet=True)
    nc.compile()
    return nc


def host_prep(padded: np.ndarray, positions: np.ndarray, n_cores: int,
              in_bf16: bool, out_bf16: bool):
    """Shard + build metadata. padded: (B, npad, npad) f32, positions: (B, 2)."""
    B, npad, _ = padded.shape
    n = npad - 2 * PAD
    bpc = B // n_cores

    px = positions[:, 0].astype(np.float32)
    py = positions[:, 1].astype(np.float32)
    fy = np.floor(py)
    fx = np.floor(px)
    ay = (5 + fy).astype(np.int64)
    ax = (5 + fx).astype(np.int64)
    wy = np.clip((py - fy).astype(np.float32), EPS, None)
    wx = np.clip((px - fx).astype(np.float32), EPS, None)

    m_lo = int(max(0, -min(ay.min(), ax.min())))
    m_hi = int(max(0, max(ay.max(), ax.max()) + (n + 1) - npad))
    wpad = npad + m_lo + m_hi

    cv = (wy * wx).astype(np.float32)
    pp = np.zeros((B, wpad, wpad), dtype=np.float32)
    # fold the wy*wx scale into the zero-pad copy (frees an on-chip op)
    pp[:, m_lo:m_lo + npad, m_lo:m_lo + npad] = \
        padded * cv[:, None, None]
    if in_bf16:
        import ml_dtypes
        pp = pp.astype(ml_dtypes.bfloat16)

    av = ((1.0 - wy) / wy).astype(np.float32)
    bv = ((1.0 - wx) / wx).astype(np.float32)
    A = ay + m_lo
    Bc = ax + m_lo

    cfg = Cfg(bpc=bpc, n=n, wpad=wpad, in_bf16=in_bf16, out_bf16=out_bf16)
    GI, PPI = cfg.gi, cfg.ppi
    NG = bpc // GI

    in_maps = []
    for cidx in range(n_cores):
        sl = slice(cidx * bpc, (cidx + 1) * bpc)
        x_flat = np.concatenate(
            [pp[sl].reshape(-1),
             np.zeros(wpad, dtype=pp.dtype)])[None, :]
        base = np.arange(bpc, dtype=np.int64) * (wpad * wpad)
        offs = (base + A[sl] * wpad + Bc[sl]).astype(np.int32)[None, :]
        wvec = np.zeros((128, 4 * NG), dtype=np.float32)
        for g in range(NG):
            for s in range(GI):
                i = cidx * bpc + g * GI + s
                prt = slice(s * PPI, (s + 1) * PPI)
                wvec[prt, 4 * g + 0] = av[i]
                wvec[prt, 4 * g + 1] = bv[i]
        in_maps.append({"x": np.ascontiguousarray(x_flat),
                        "offs": offs, "wvec": wvec})
    return cfg, in_maps


IN_BF16 = True
OUT_BF16 = True
_nc_cache: dict = {}


def kernel(padded_obj: np.ndarray, positions: np.ndarray) -> np.ndarray:
    padded_obj = np.asarray(padded_obj)
    positions = np.asarray(positions)
    B, npad, _, C = padded_obj.shape
    cfg, in_maps = host_prep(
        padded_obj.reshape(B, npad, npad).astype(np.float32, copy=False),
        positions, N_CORES, IN_BF16, OUT_BF16)

    nc = _nc_cache.get(cfg)
    if nc is None:
        nc = build_nc(cfg)
        _nc_cache[cfg] = nc

    res = run_bass_kernel_spmd(nc, in_maps, core_ids=list(range(N_CORES)))
    out = np.concatenate(
        [np.asarray(r["y"]).astype(np.float32).reshape(cfg.bpc, cfg.n, cfg.n)
         for r in res.results], axis=0)
    return out.reshape(B, cfg.n, cfg.n, 1)
